# revision 15
# baseline (speedup 1.0000x reference)
"""ChebNet (magnetic-Laplacian Chebyshev GNN) Trainium2 kernel.

Runs data-parallel over 8 NeuronCores: batch 2048 -> 256 per core.
The two ChebConv layers + CVLinear(64->1) are algebraically collapsed into a
degree-6 Chebyshev polynomial of the Laplacian applied to per-batch vectors
(product identity T_j T_k = (T_{j+k} + T_{|j-k|})/2), evaluated with the
Clenshaw recurrence on VectorE.  The only cross-core traffic is the complex
BatchNorm statistics all-reduce (750 floats).
"""

import math
from contextlib import ExitStack

import numpy as np

import concourse.bass as bass
import concourse.bacc as bacc
import concourse.tile as tile
import concourse.mybir as mybir
from concourse import masks
from concourse.bass_utils import run_bass_kernel_spmd

# Problem constants (hardcoded per harness contract)
B, BANDS, N, INC, F, K, Q, OUT = 2048, 5, 30, 5, 64, 4, 0.25, 9
EPS = 1e-5
NCORES = 8
BS = B // NCORES          # 256 instances per core
PT = 128                  # partition tile
NT = BS // PT             # 2 partition-tiles per core
NN = N * N                # 900
DT = mybir.dt.float32
AX = mybir.AxisListType
OP = mybir.AluOpType
AF = mybir.ActivationFunctionType

# s-stage grouping: output nodes split into 4 groups, two per transposed half
GROUPS = [(0, 8), (8, 15), (15, 23), (23, 30)]  # [start, end) node ranges
CAUG = INC + 1            # x augmented with a ones column -> folds g_m * 1


def _build_poly(layer, c1W, c1b, c2W, c2b, lWr, lWi, lbr, lbi):
    """Fold both cheb layers + lin into T-basis coefficients E [M,INC], g [M]."""
    Wl = (lWr + 1j * lWi)[:, 0].astype(complex)
    bl = complex(lbr[0]) + 1j * complex(lbi[0])
    if layer > 1:
        c2 = np.einsum('kfg,g->kf', c2W.astype(np.float64), Wl)
        d = np.einsum('jcf,kf->jkc', c1W.astype(np.float64), c2)
        M = 2 * K - 1
        E = np.zeros((M, INC), complex)
        g = np.zeros(M, complex)
        for j in range(K):
            for k in range(K):
                E[j + k] += 0.5 * d[j, k]
                E[abs(j - k)] += 0.5 * d[j, k]
        beta = (1 + 1j) * c1b.astype(np.float64)
        for k in range(K):
            g[k] += beta @ c2[k]
        g[0] += (1 + 1j) * (c2b.astype(np.float64) @ Wl) + bl
    else:
        M = K
        E = np.einsum('jcf,f->jc', c1W.astype(np.float64), Wl)
        g = np.zeros(M, complex)
        g[0] += (1 + 1j) * (c1b.astype(np.float64) @ Wl) + bl
    return E, g


def _build_eqcat(E, g):
    """lhsT blocks for the s-stage matmuls (PSUM-accumulated).

    Per group G: xr-block [gn*INC, cols] and xi-block at col-offset maxcols;
    col order (n-g0)*2M + ri_out*M + m.  Returns (eqcat [40, 8*maxcols],
    gq [1, 4*maxcols]) where gq carries the constant g_m contribution.
    """
    M = E.shape[0]
    Er = E.real.astype(np.float32)   # [M, INC]
    Ei = E.imag.astype(np.float32)
    # C[ri_in, c, ri_out, m]
    C = np.zeros((2, INC, 2, M), np.float32)
    C[0, :, 0, :] = Er.T
    C[0, :, 1, :] = Ei.T
    C[1, :, 0, :] = -Ei.T
    C[1, :, 1, :] = Er.T
    maxcols = 8 * 2 * M
    eqcat = np.zeros((8 * INC, 2 * 4 * maxcols), np.float32)
    gq = np.zeros((1, 4 * maxcols), np.float32)
    for gi, (g0, g1) in enumerate(GROUPS):
        gn = g1 - g0
        base = gi * 2 * maxcols
        for ri_in in range(2):
            for nn_ in range(gn):
                for c in range(INC):
                    r = nn_ * INC + c
                    col0 = base + ri_in * maxcols + nn_ * 2 * M
                    eqcat[r, col0: col0 + 2 * M] = C[ri_in, c].reshape(2 * M)
        for nn_ in range(gn):
            for ri_o in range(2):
                for m in range(M):
                    val = g.real[m] if ri_o == 0 else g.imag[m]
                    gq[0, gi * maxcols + nn_ * 2 * M + ri_o * M + m] = val
    return eqcat, gq


def _build_fcw(fc_Wr, fc_Wi, fc_br, fc_bi):
    """fc lhsT [30, 54]: cols 0:18 hr-block, 18:36 hi-block, 36:54 bias row."""
    w = np.zeros((N, 3 * 2 * OUT), np.float32)
    w[:, 0:2 * OUT:2] = fc_Wr
    w[:, 1:2 * OUT:2] = fc_Wi
    w[:, 2 * OUT:4 * OUT:2] = -fc_Wi
    w[:, 2 * OUT + 1:4 * OUT:2] = fc_Wr
    w[0, 4 * OUT::2] = fc_br
    w[0, 4 * OUT + 1::2] = fc_bi
    return w


def _build_nc(layer):
    """Build the per-core Bass program (same NEFF on all 8 cores)."""
    M = 2 * K - 1 if layer > 1 else K
    M2 = 2 * M
    eq_maxcols = 8 * M2

    nc = bacc.Bacc("TRN2", target_bir_lowering=False, debug=False,
                   num_devices=NCORES)
    g_graph = nc.declare_dram_parameter("graph", [BS, BANDS, N, N], DT, isOutput=False)
    g_xr = nc.declare_dram_parameter("xr", [BS, N, INC], DT, isOutput=False)
    g_xi = nc.declare_dram_parameter("xi", [BS, N, INC], DT, isOutput=False)
    g_eq = nc.declare_dram_parameter("eqcat", [8 * INC, 8 * eq_maxcols], DT, isOutput=False)
    g_gq = nc.declare_dram_parameter("gq", [1, 4 * eq_maxcols], DT, isOutput=False)
    g_fcw = nc.declare_dram_parameter("fcw", [N, 3 * 2 * OUT], DT, isOutput=False)
    g_bnc = nc.declare_dram_parameter("bnc", [5 * N], DT, isOutput=False)
    g_out = nc.declare_dram_parameter("out", [BS, BANDS, OUT, 2], DT, isOutput=True)

    bnp = nc.dram_tensor("bnp", [750], DT)
    bnr = nc.dram_tensor("bnr", [750], DT, addr_space="Shared")

    TWO_PI_Q = 2.0 * math.pi * Q

    # register pi/2 const AP (cos x = sin(x + pi/2) activation bias)
    half_pi = math.pi / 2
    ctile = nc.alloc_sbuf_tensor("const-f32-halfpi", [128, 1], DT)
    nc.gpsimd.memset(ctile.ap(), half_pi)
    nc.const_aps.aps[(DT, half_pi)] = ctile.ap()

    with tile.TileContext(nc) as tc, ExitStack() as ctx:
        const = ctx.enter_context(tc.tile_pool(name="const", bufs=1))
        xpool = ctx.enter_context(tc.tile_pool(name="x", bufs=2))
        spool = ctx.enter_context(tc.tile_pool(name="s", bufs=2))
        apool = ctx.enter_context(tc.tile_pool(name="a", bufs=3))
        wpool = ctx.enter_context(tc.tile_pool(name="w", bufs=2))
        lpool = ctx.enter_context(tc.tile_pool(name="l", bufs=2))
        mpool = ctx.enter_context(tc.tile_pool(name="m", bufs=2))
        bpool = ctx.enter_context(tc.tile_pool(name="b", bufs=4))
        fpool = ctx.enter_context(tc.tile_pool(name="f", bufs=2))
        gpool = ctx.enter_context(tc.tile_pool(name="g", bufs=1))
        vpool = ctx.enter_context(tc.tile_pool(name="v", bufs=2))
        opool = ctx.enter_context(tc.tile_pool(name="o", bufs=2))
        pst = ctx.enter_context(tc.tile_pool(name="pst", bufs=1, space="PSUM"))
        psm = ctx.enter_context(tc.tile_pool(name="psm", bufs=1, space="PSUM"))
        psb = ctx.enter_context(tc.tile_pool(name="psb", bufs=1, space="PSUM"))

        # ---- constants ----
        eqt = const.tile([8 * INC, 8 * eq_maxcols], DT, tag="eq")
        nc.sync.dma_start(eqt[:], g_eq[:, :])
        gqt = const.tile([1, 4 * eq_maxcols], DT, tag="gq")
        nc.sync.dma_start(gqt[:], g_gq[:, :])
        fcwt = const.tile([N, 3 * 2 * OUT], DT, tag="fcw")
        nc.sync.dma_start(fcwt[:], g_fcw[:, :])
        bnct = const.tile([1, 5 * N], DT, tag="bnc")
        nc.sync.dma_start(bnct[:], g_bnc[:].rearrange("(a b) -> a b", a=1, b=5 * N))
        ident = const.tile([PT, PT], DT, tag="id")
        masks.make_identity(nc, ident[:])
        ones_col = const.tile([PT, 1], DT, tag="ones_col")
        nc.vector.memset(ones_col[:], 1.0)
        ones_row = const.tile([1, PT], DT, tag="ones_row")
        nc.vector.memset(ones_row[:], 1.0)

        # ---- stage s: s_m = X @ e_m + g_m (PE, PSUM-accumulated) ----
        stiles = []
        stacks_v = []  # stat stacks per tile
        for t in range(NT):
            b0 = t * PT
            xr_t = xpool.tile([PT, N * INC], DT, tag="xr")
            xi_t = xpool.tile([PT, N * INC], DT, tag="xi")
            nc.sync.dma_start(
                xr_t[:].rearrange("p (n c) -> p n c", n=N, c=INC),
                g_xr[b0:b0 + PT, :, :])
            nc.sync.dma_start(
                xi_t[:].rearrange("p (n c) -> p n c", n=N, c=INC),
                g_xi[b0:b0 + PT, :, :])

            stile = spool.tile([PT, N * M2], DT, tag="s")
            stiles.append(stile)

            for gi, (g0, g1) in enumerate(GROUPS):
                gn = g1 - g0
                rows = gn * INC
                base = gi * 2 * eq_maxcols
                txr = pst.tile([8 * INC, PT], DT, tag="txr")
                nc.tensor.transpose(txr[0:rows, :],
                                    xr_t[:, g0 * INC:g1 * INC], ident[:])
                txi = pst.tile([8 * INC, PT], DT, tag="txi")
                nc.tensor.transpose(txi[0:rows, :],
                                    xi_t[:, g0 * INC:g1 * INC], ident[:])
                txr_s = wpool.tile([8 * INC, PT], DT, tag="txr_s")
                nc.scalar.copy(txr_s[0:rows, :], txr[0:rows, :])
                txi_s = wpool.tile([8 * INC, PT], DT, tag="txi_s")
                nc.scalar.copy(txi_s[0:rows, :], txi[0:rows, :])
                cols = gn * M2
                mm = psm.tile([eq_maxcols, PT], DT, tag="smm")
                nc.tensor.matmul(mm[0:cols, :], eqt[0:rows, base: base + cols],
                                 txr_s[0:rows, :], start=True, stop=False)
                nc.tensor.matmul(
                    mm[0:cols, :],
                    eqt[0:rows, base + eq_maxcols: base + eq_maxcols + cols],
                    txi_s[0:rows, :], start=False, stop=False)
                nc.tensor.matmul(
                    mm[0:cols, :],
                    gqt[0:1, gi * eq_maxcols: gi * eq_maxcols + cols],
                    ones_row[:], start=False, stop=True)
                mm_s = wpool.tile([eq_maxcols, PT], DT, tag="smm_s")
                nc.scalar.copy(mm_s[0:cols, :], mm[0:cols, :])
                tb = psm.tile([PT, eq_maxcols], DT, tag="stb")
                nc.tensor.transpose(tb[:, 0:cols], mm_s[0:cols, :],
                                    ident[0:cols, 0:cols])
                nc.scalar.copy(stile[:, g0 * M2: g0 * M2 + cols], tb[:, 0:cols])

            stack_v = vpool.tile([PT, 750], DT, tag="stk")
            stacks_v.append(stack_v)

        # ---- per (tile, band): build L, Clenshaw ----
        for t in range(NT):
            b0 = t * PT
            stile = stiles[t]
            s3 = stile[:].rearrange("p (n q) -> p q n", n=N, q=M2)
            stack_v = stacks_v[t]

            def s_sl(m, ri, s3=s3):
                return s3[:, ri * M + m, :]

            for band in range(BANDS):
                at = apool.tile([PT, NN], DT, tag="at")
                nc.sync.dma_start(
                    at[:].rearrange("p (i j) -> p i j", i=N, j=N),
                    g_graph[b0:b0 + PT, band, :, :])
                a3 = at[:].rearrange("p (i j) -> p i j", i=N, j=N)
                a3T = at[:].rearrange("p (i j) -> p j i", i=N, j=N)

                St = wpool.tile([PT, NN], DT, tag="S")
                S3 = St[:].rearrange("p (i j) -> p i j", i=N, j=N)
                nc.vector.tensor_tensor(S3, a3, a3T, OP.add)
                Dt = wpool.tile([PT, NN], DT, tag="D")
                nc.vector.tensor_tensor(
                    Dt[:].rearrange("p (i j) -> p i j", i=N, j=N), a3, a3T,
                    OP.subtract)

                row = bpool.tile([PT, N], DT, tag="row")
                nc.vector.tensor_reduce(row[:], S3, AX.X, OP.add)
                td = bpool.tile([PT, N], DT, tag="td")
                nc.scalar.activation(td[:], row[:], AF.Sqrt, bias=1.0, scale=0.5)
                dd = bpool.tile([PT, N], DT, tag="dd")
                nc.vector.reciprocal(dd[:], td[:])     # d = 1/sqrt(0.5 row + 1)
                da = bpool.tile([PT, N], DT, tag="da")
                nc.vector.tensor_scalar_mul(da[:], dd[:], -0.5)

                # M = -0.5 * S * d_i * d_j
                P1 = wpool.tile([PT, NN], DT, tag="P1")
                da_b = da[:].to_broadcast([PT, N, N])              # (i:1, j:0)
                nc.vector.tensor_tensor(
                    P1[:].rearrange("p (i j) -> p i j", i=N, j=N), S3, da_b,
                    OP.mult)
                Mt = wpool.tile([PT, NN], DT, tag="Mt")
                d_bj = dd[:].to_broadcast([PT, N, N]).rearrange("p a b -> p b a")
                nc.vector.tensor_tensor(
                    Mt[:].rearrange("p (i j) -> p i j", i=N, j=N),
                    P1[:].rearrange("p (i j) -> p i j", i=N, j=N), d_bj, OP.mult)

                Ct = wpool.tile([PT, NN], DT, tag="Ct")
                nc.scalar.activation(Ct[:], Dt[:], AF.Sin, bias=math.pi / 2,
                                     scale=TWO_PI_Q)
                Snt = wpool.tile([PT, NN], DT, tag="Snt")
                nc.scalar.activation(Snt[:], Dt[:], AF.Sin, bias=0.0,
                                     scale=TWO_PI_Q)

                Lr = lpool.tile([PT, NN], DT, tag="Lr")
                nc.vector.tensor_tensor(Lr[:], Mt[:], Ct[:], OP.mult)
                Li = lpool.tile([PT, NN], DT, tag="Li")
                nc.vector.tensor_tensor(Li[:], Mt[:], Snt[:], OP.mult)

                # diag correction: Lr[ii] -= d_i^2
                dsq = bpool.tile([PT, N], DT, tag="dsq")
                nc.vector.tensor_tensor(dsq[:], dd[:], dd[:], OP.mult)
                nc.vector.tensor_tensor(Lr[:, ::N + 1], Lr[:, ::N + 1], dsq[:],
                                        OP.subtract)
                Ls = lpool.tile([PT, NN], DT, tag="Ls")
                nc.vector.tensor_tensor(Ls[:], Lr[:], Li[:], OP.add)

                Lr3 = Lr[:].rearrange("p (i j) -> p i j", i=N, j=N)
                Li3 = Li[:].rearrange("p (i j) -> p i j", i=N, j=N)
                Ls3 = Ls[:].rearrange("p (i j) -> p i j", i=N, j=N)

                def bcast(ap):
                    return ap.to_broadcast([PT, N, N]).rearrange("p a b -> p b a")

                # Clenshaw: b_k = s_k + 2 L b_{k+1} - b_{k+2};  v = s_0 + L b_1 - b_2
                b1r = s_sl(M - 1, 0)
                b1i = s_sl(M - 1, 1)
                b2r = None  # zero
                b2i = None
                for k in range(M - 2, -1, -1):
                    hs = bpool.tile([PT, N], DT, tag="hs")
                    nc.vector.tensor_tensor(hs[:], b1r, b1i, OP.add)
                    mt = mpool.tile([PT, 3 * NN], DT, tag="mt")
                    nc.vector.tensor_tensor(
                        mt[:, 0:NN].rearrange("p (i j) -> p i j", i=N, j=N),
                        Lr3, bcast(b1r), OP.mult)
                    nc.vector.tensor_tensor(
                        mt[:, NN:2 * NN].rearrange("p (i j) -> p i j", i=N, j=N),
                        Li3, bcast(b1i), OP.mult)
                    nc.vector.tensor_tensor(
                        mt[:, 2 * NN:3 * NN].rearrange("p (i j) -> p i j", i=N, j=N),
                        Ls3, bcast(hs[:]), OP.mult)
                    r = bpool.tile([PT, 3 * N], DT, tag="r")
                    nc.vector.tensor_reduce(
                        r[:], mt[:].rearrange("p (x j) -> p x j", x=3 * N, j=N),
                        AX.X, OP.add)
                    r1, r2, r3 = r[:, 0:N], r[:, N:2 * N], r[:, 2 * N:3 * N]
                    fac = 2.0 if k > 0 else 1.0
                    is_final = (k == 0)

                    t1 = bpool.tile([PT, N], DT, tag="t1")
                    nc.vector.tensor_tensor(t1[:], r1, r2, OP.subtract)
                    t2 = bpool.tile([PT, N], DT, tag="t2")
                    nc.vector.tensor_tensor(t2[:], r3, r1, OP.subtract)
                    nc.vector.tensor_tensor(t2[:], t2[:], r2, OP.subtract)

                    if is_final:
                        nbr = stack_v[:, band * N:(band + 1) * N]
                        nbi = stack_v[:, 150 + band * N: 150 + (band + 1) * N]
                    else:
                        nbr = bpool.tile([PT, N], DT, tag="nbr", name="nbr")[:]
                        nbi = bpool.tile([PT, N], DT, tag="nbi", name="nbi")[:]
                    if b2r is None:
                        if fac != 1.0:
                            nc.vector.tensor_scalar(t1[:], t1[:], fac, None, OP.mult)
                            nc.vector.tensor_scalar(t2[:], t2[:], fac, None, OP.mult)
                        nc.vector.tensor_tensor(nbr, t1[:], s_sl(k, 0), OP.add)
                        nc.vector.tensor_tensor(nbi, t2[:], s_sl(k, 1), OP.add)
                    else:
                        nc.vector.scalar_tensor_tensor(t1[:], t1[:], fac, b2r,
                                                       OP.mult, OP.subtract)
                        nc.vector.tensor_tensor(nbr, t1[:], s_sl(k, 0), OP.add)
                        nc.vector.scalar_tensor_tensor(t2[:], t2[:], fac, b2i,
                                                       OP.mult, OP.subtract)
                        nc.vector.tensor_tensor(nbi, t2[:], s_sl(k, 1), OP.add)
                    b2r, b2i = b1r, b1i
                    b1r, b1i = nbr, nbi

        # ---- BN stats: per-core partial sums -> AllReduce ----
        bn_ps0 = psb.tile([1, 512], DT, tag="bn0")
        bn_ps1 = psb.tile([1, 238], DT, tag="bn1")
        for t in range(NT):
            stack_v = stacks_v[t]
            vr = stack_v[:, 0:150]
            vi = stack_v[:, 150:300]
            nc.vector.tensor_tensor(stack_v[:, 300:450], vr, vr, OP.mult)
            nc.vector.tensor_tensor(stack_v[:, 450:600], vi, vi, OP.mult)
            nc.vector.tensor_tensor(stack_v[:, 600:750], vr, vi, OP.mult)
            nc.tensor.matmul(bn_ps0[:], ones_col[:], stack_v[:, 0:512],
                             start=(t == 0), stop=(t == NT - 1))
            nc.tensor.matmul(bn_ps1[:], ones_col[:], stack_v[:, 512:750],
                             start=(t == 0), stop=(t == NT - 1))
        bn_sb = gpool.tile([1, 750], DT, tag="bnsb")
        nc.scalar.copy(bn_sb[:, 0:512], bn_ps0[:])
        nc.scalar.copy(bn_sb[:, 512:750], bn_ps1[:])
        nc.sync.dma_start(bnp[:].rearrange("(a b) -> a b", a=1, b=750), bn_sb[:])
        nc.gpsimd.collective_compute(
            "AllReduce", OP.add, replica_groups=[list(range(NCORES))],
            ins=[bnp[:]], outs=[bnr[:]])
        bn = gpool.tile([1, 750], DT, tag="bn")
        nc.sync.dma_start(bn[:], bnr[:].rearrange("(a b) -> a b", a=1, b=750))

        # ---- finish BN stats on partition 0 ----
        def g1tile(tag):
            return gpool.tile([1, 150], DT, tag=tag, name=tag)

        cs = gpool.tile([1, 1200], DT, tag="cs")   # coeff source
        mr, mi = cs[:, 0:150], cs[:, 150:300]
        inv_b = 1.0 / float(B)
        nc.vector.tensor_scalar(mr, bn[:, 0:150], inv_b, None, OP.mult)
        nc.vector.tensor_scalar(mi, bn[:, 150:300], inv_b, None, OP.mult)
        e2r, e2i, eri = g1tile("e2r"), g1tile("e2i"), g1tile("eri")
        nc.vector.tensor_scalar(e2r[:], bn[:, 300:450], inv_b, None, OP.mult)
        nc.vector.tensor_scalar(e2i[:], bn[:, 450:600], inv_b, None, OP.mult)
        nc.vector.tensor_scalar(eri[:], bn[:, 600:750], inv_b, None, OP.mult)
        sq, Vrr, Vii, Vri = g1tile("sq"), g1tile("Vrr"), g1tile("Vii"), g1tile("Vri")
        nc.vector.tensor_tensor(sq[:], mr, mr, OP.mult)
        nc.vector.tensor_tensor(Vrr[:], e2r[:], sq[:], OP.subtract)
        nc.vector.tensor_scalar(Vrr[:], Vrr[:], EPS, None, OP.add)
        nc.vector.tensor_tensor(sq[:], mi, mi, OP.mult)
        nc.vector.tensor_tensor(Vii[:], e2i[:], sq[:], OP.subtract)
        nc.vector.tensor_scalar(Vii[:], Vii[:], EPS, None, OP.add)
        nc.vector.tensor_tensor(sq[:], mr, mi, OP.mult)
        nc.vector.tensor_tensor(Vri[:], eri[:], sq[:], OP.subtract)
        det, sdet, tn, inv = g1tile("det"), g1tile("sdet"), g1tile("tn"), g1tile("inv")
        nc.vector.tensor_tensor(det[:], Vrr[:], Vii[:], OP.mult)
        nc.vector.tensor_tensor(sq[:], Vri[:], Vri[:], OP.mult)
        nc.vector.tensor_tensor(det[:], det[:], sq[:], OP.subtract)
        nc.scalar.activation(sdet[:], det[:], AF.Sqrt)
        nc.vector.tensor_tensor(tn[:], Vrr[:], Vii[:], OP.add)
        nc.vector.scalar_tensor_tensor(tn[:], sdet[:], 2.0, tn[:], OP.mult, OP.add)
        nc.scalar.activation(tn[:], tn[:], AF.Sqrt)
        nc.vector.tensor_tensor(tn[:], sdet[:], tn[:], OP.mult)
        nc.vector.reciprocal(inv[:], tn[:])
        Wrr, Wii, Wri = g1tile("Wrr"), g1tile("Wii"), g1tile("Wri")
        nc.vector.tensor_tensor(Wrr[:], Vii[:], sdet[:], OP.add)
        nc.vector.tensor_tensor(Wrr[:], Wrr[:], inv[:], OP.mult)
        nc.vector.tensor_tensor(Wii[:], Vrr[:], sdet[:], OP.add)
        nc.vector.tensor_tensor(Wii[:], Wii[:], inv[:], OP.mult)
        nc.vector.tensor_scalar(sq[:], Vri[:], -1.0, None, OP.mult)
        nc.vector.tensor_tensor(Wri[:], sq[:], inv[:], OP.mult)

        def bnc_b(i):
            """gamma/beta [N] -> (1, BANDS, N) broadcast view."""
            return (bnct[:, i * N:(i + 1) * N].to_broadcast([1, N, BANDS])
                    .rearrange("p a b -> p b a"))

        def c5(apx):
            return apx.rearrange("p (b n) -> p b n", b=BANDS, n=N)

        P11, P12 = cs[:, 300:450], cs[:, 450:600]
        P21, P22 = cs[:, 600:750], cs[:, 750:900]
        bb1, bb2 = cs[:, 900:1050], cs[:, 1050:1200]
        tmp = g1tile("tmpc")
        nc.vector.tensor_tensor(c5(P11), bnc_b(0), c5(Wrr[:]), OP.mult)
        nc.vector.tensor_tensor(c5(tmp[:]), bnc_b(1), c5(Wri[:]), OP.mult)
        nc.vector.tensor_tensor(P11, P11, tmp[:], OP.add)
        nc.vector.tensor_tensor(c5(P12), bnc_b(0), c5(Wri[:]), OP.mult)
        nc.vector.tensor_tensor(c5(tmp[:]), bnc_b(1), c5(Wii[:]), OP.mult)
        nc.vector.tensor_tensor(P12, P12, tmp[:], OP.add)
        nc.vector.tensor_tensor(c5(P21), bnc_b(1), c5(Wrr[:]), OP.mult)
        nc.vector.tensor_tensor(c5(tmp[:]), bnc_b(2), c5(Wri[:]), OP.mult)
        nc.vector.tensor_tensor(P21, P21, tmp[:], OP.add)
        nc.vector.tensor_tensor(c5(P22), bnc_b(1), c5(Wri[:]), OP.mult)
        nc.vector.tensor_tensor(c5(tmp[:]), bnc_b(2), c5(Wii[:]), OP.mult)
        nc.vector.tensor_tensor(P22, P22, tmp[:], OP.add)
        nc.vector.tensor_copy(c5(bb1), bnc_b(3))
        nc.vector.tensor_copy(c5(bb2), bnc_b(4))

        # broadcast coeffs to 128 partitions via PE
        bcoef = const.tile([PT, 1200], DT, tag="bcoef")
        for i in range(3):
            pb = psb.tile([PT, 400], DT, tag="pb")
            nc.tensor.matmul(pb[:], ones_row[:], cs[:, i * 400:(i + 1) * 400],
                             start=True, stop=True)
            nc.scalar.copy(bcoef[:, i * 400:(i + 1) * 400], pb[:])

        # ---- BN apply + polar tanh + fc + out ----
        for t in range(NT):
            b0 = t * PT
            stack_v = stacks_v[t]
            cr = fpool.tile([PT, 150], DT, tag="cr")
            ci = fpool.tile([PT, 150], DT, tag="ci")
            nc.vector.tensor_tensor(cr[:], stack_v[:, 0:150], bcoef[:, 0:150],
                                    OP.subtract)
            nc.vector.tensor_tensor(ci[:], stack_v[:, 150:300], bcoef[:, 150:300],
                                    OP.subtract)
            xr_ = fpool.tile([PT, 150], DT, tag="xr_")
            xi_ = fpool.tile([PT, 150], DT, tag="xi_")
            tmp2 = fpool.tile([PT, 150], DT, tag="tmp2")
            nc.vector.tensor_tensor(xr_[:], cr[:], bcoef[:, 300:450], OP.mult)
            nc.vector.tensor_tensor(tmp2[:], ci[:], bcoef[:, 450:600], OP.mult)
            nc.vector.tensor_tensor(xr_[:], xr_[:], tmp2[:], OP.add)
            nc.vector.tensor_tensor(xr_[:], xr_[:], bcoef[:, 900:1050], OP.add)
            nc.vector.tensor_tensor(xi_[:], cr[:], bcoef[:, 600:750], OP.mult)
            nc.vector.tensor_tensor(tmp2[:], ci[:], bcoef[:, 750:900], OP.mult)
            nc.vector.tensor_tensor(xi_[:], xi_[:], tmp2[:], OP.add)
            nc.vector.tensor_tensor(xi_[:], xi_[:], bcoef[:, 1050:1200], OP.add)
            # polar tanh
            msq = fpool.tile([PT, 150], DT, tag="msq")
            nc.vector.tensor_tensor(msq[:], xr_[:], xr_[:], OP.mult)
            nc.vector.tensor_tensor(tmp2[:], xi_[:], xi_[:], OP.mult)
            nc.vector.tensor_tensor(msq[:], msq[:], tmp2[:], OP.add)
            mag = fpool.tile([PT, 150], DT, tag="mag")
            nc.scalar.activation(mag[:], msq[:], AF.Sqrt)
            th = fpool.tile([PT, 150], DT, tag="th")
            nc.scalar.activation(th[:], mag[:], AF.Tanh)
            nc.vector.tensor_scalar(mag[:], mag[:], 1e-12, None, OP.max)
            rec = fpool.tile([PT, 150], DT, tag="rec")
            nc.vector.reciprocal(rec[:], mag[:])
            nc.vector.tensor_tensor(rec[:], rec[:], th[:], OP.mult)
            hr = fpool.tile([PT, 150], DT, tag="hr")
            hi = fpool.tile([PT, 150], DT, tag="hi")
            nc.vector.tensor_tensor(hr[:], xr_[:], rec[:], OP.mult)
            nc.vector.tensor_tensor(hi[:], xi_[:], rec[:], OP.mult)

            otile = opool.tile([PT, BANDS * OUT * 2], DT, tag="ot")
            for band in range(BANDS):
                thr = pst.tile([N, PT], DT, tag="txr")
                nc.tensor.transpose(thr[:], hr[:, band * N:(band + 1) * N], ident[:])
                thi = pst.tile([N, PT], DT, tag="txi")
                nc.tensor.transpose(thi[:], hi[:, band * N:(band + 1) * N], ident[:])
                thr_s = wpool.tile([N, PT], DT, tag="thr_s")
                nc.scalar.copy(thr_s[:], thr[:])
                thi_s = wpool.tile([N, PT], DT, tag="thi_s")
                nc.scalar.copy(thi_s[:], thi[:])
                fco = psm.tile([2 * OUT, PT], DT, tag="smm")
                nc.tensor.matmul(fco[:], fcwt[:, 0:2 * OUT], thr_s[:],
                                 start=True, stop=False)
                nc.tensor.matmul(fco[:], fcwt[:, 2 * OUT:4 * OUT], thi_s[:],
                                 start=False, stop=False)
                nc.tensor.matmul(fco[:], fcwt[0:1, 4 * OUT:6 * OUT], ones_row[:],
                                 start=False, stop=True)
                fcs = wpool.tile([2 * OUT, PT], DT, tag="fcs")
                nc.scalar.copy(fcs[:], fco[:])
                tout = psm.tile([PT, 2 * OUT], DT, tag="stb")
                nc.tensor.transpose(tout[:], fcs[:], ident[0:2 * OUT, 0:2 * OUT])
                nc.scalar.copy(otile[:, band * 2 * OUT:(band + 1) * 2 * OUT], tout[:])
            nc.sync.dma_start(
                g_out[b0:b0 + PT, :, :, :],
                otile[:].rearrange("p (b o r) -> p b o r", b=BANDS, o=OUT, r=2))
    nc.compile()
    return nc


LAST_IN_MAPS = None    # most recent per-core input maps (for benching)

_CACHE = {}


def _get_nc(layer):
    if layer not in _CACHE:
        _CACHE[layer] = _build_nc(layer)
    return _CACHE[layer]


def kernel(**inputs):
    real = np.ascontiguousarray(np.asarray(inputs["real"], np.float32))
    imag = np.ascontiguousarray(np.asarray(inputs["imag"], np.float32))
    graph = np.ascontiguousarray(np.asarray(inputs["graph"], np.float32))
    layer = int(np.asarray(inputs["layer"]))

    E, g = _build_poly(
        layer,
        np.asarray(inputs["cheb1_W"], np.float32), np.asarray(inputs["cheb1_b"], np.float32),
        np.asarray(inputs["cheb2_W"], np.float32), np.asarray(inputs["cheb2_b"], np.float32),
        np.asarray(inputs["lin_Wr"], np.float32), np.asarray(inputs["lin_Wi"], np.float32),
        np.asarray(inputs["lin_br"], np.float32), np.asarray(inputs["lin_bi"], np.float32))
    eqcat, gq = _build_eqcat(E, g)
    fcw = _build_fcw(
        np.asarray(inputs["fc_Wr"], np.float32), np.asarray(inputs["fc_Wi"], np.float32),
        np.asarray(inputs["fc_br"], np.float32), np.asarray(inputs["fc_bi"], np.float32))
    bnc = np.concatenate([
        np.asarray(inputs["gamma_rr"], np.float32),
        np.asarray(inputs["gamma_ri"], np.float32),
        np.asarray(inputs["gamma_ii"], np.float32),
        np.asarray(inputs["beta_r"], np.float32),
        np.asarray(inputs["beta_i"], np.float32)])
    nc = _get_nc(layer)
    in_maps = []
    for c in range(NCORES):
        sl = slice(c * BS, (c + 1) * BS)
        in_maps.append({
            "graph": np.ascontiguousarray(graph[sl]),
            "xr": np.ascontiguousarray(real[sl]),
            "xi": np.ascontiguousarray(imag[sl]),
            "eqcat": eqcat, "gq": gq, "fcw": fcw, "bnc": bnc,
        })
    global LAST_IN_MAPS
    LAST_IN_MAPS = in_maps
    res = run_bass_kernel_spmd(nc, in_maps, core_ids=list(range(NCORES)))
    out = np.concatenate([res.results[i]["out"] for i in range(NCORES)], axis=0)
    return out.astype(np.float32)
